# revision 48
# baseline (speedup 1.0000x reference)
import sys
sys.path.insert(0, "/opt/trn_rl_repo")
import numpy as np
from contextlib import ExitStack

from concourse import bacc, mybir, tile
from concourse import bass_utils
from concourse.masks import make_identity

# nn_MultiHeadAttention: B=4, T=2048, C=1024, H=16, HS=64
# Sharding: core = 2*b + hh; each core handles batch b, heads hh*8..hh*8+7.
# Per-core output is a partial [T, C] (its 8 heads through Wproj rows);
# host sums the pair (hh=0,1) per batch. Bias folded into hh==0 cores.
#
# v7 (from v2 baseline):
#  - l (softmax denominator) fused into the PV matmul: stationary widened
#    to [v_h | ones] so PSUM rows 64-127 accumulate the row-sum while rows
#    0-63 accumulate P@V. Matmul cost is moving-column count, so l is free;
#    v2's separate ones-stationary l matmuls are gone.
#  - S = K^T Q in fp8 DoubleRow: q/k tiles quantized to fp8 on the PSUM
#    drain, then SWDGE SB->SB DMAs remap [par*64+dh*32+q] partitions into
#    [par*32+q, dh] (k-tile pairs in the free dim) as DoubleRow requires.
#  - x arrives pre-transposed (and pre-cast to fp8) from the host: no XBAR
#    transposes, no on-device fp8 casts of x.
#  - causal mask built on-device (one affine_select tril; the mask is only
#    ever applied to the 128-wide diagonal block, where it is j-independent).
#  - emission is tc4-round-major and software-pipelined: engines run their
#    streams in order, so independent PE work (next round's qk/v prep,
#    finished rounds' output proj) is spread as small fill items drawn
#    after each attention key-tile to fill the exp-wait stalls.

B, T, C = 4, 2048, 1024
H, HS = 16, 64
HL = 8            # local heads per core
NP = HL // 2      # head pairs per core
W = HL * HS       # 512
SCALE = 1.0 / 32.0  # C ** -0.5

FP8_QK = True     # x8/wqk8 -> q,k projection path
FP8_S = True      # fp8 DoubleRow for S = K^T Q (kq stored fp8)

F32 = mybir.dt.float32
F32R = mybir.dt.float32r
BF16 = mybir.dt.bfloat16
FP8 = mybir.dt.float8e4
DRMODE = mybir.MatmulPerfMode.DoubleRow
AF = mybir.ActivationFunctionType
ALU = mybir.AluOpType

_NC = {}


def _build(repeat=1):
    exp_scal = SCALE / (256.0 if FP8_QK else 1.0)

    nc = bacc.Bacc("TRN2", target_bir_lowering=False, debug=False, num_devices=1)
    # x^T, host-pretransposed: [C, T] in bf16 (v path) and fp8 (qk path)
    xt_ap = nc.dram_tensor("XT", (C, T), BF16, kind="ExternalInput").ap()
    xt8_ap = nc.dram_tensor("XT8", (C, T), FP8, kind="ExternalInput").ap()
    # WQK [pair, kq, C, 128]; kq=0 -> [Wk_even|Wk_odd], kq=1 -> [Wq_even|Wq_odd]
    wqk_ap = nc.dram_tensor("WQK", (NP * 2 * C, 128),
                            FP8 if FP8_QK else BF16,
                            kind="ExternalInput").ap()
    wv_ap = nc.dram_tensor("WV", (C, W), BF16, kind="ExternalInput").ap()
    wp_ap = nc.dram_tensor("WP", (W, C), BF16, kind="ExternalInput").ap()
    bias_ap = nc.dram_tensor("BIAS", (1, C), F32, kind="ExternalInput").ap()
    out_ap = nc.dram_tensor("OUT", (T, C), F32, kind="ExternalOutput").ap()

    xt_r = xt_ap.rearrange("(ct p) t -> p ct t", p=128)
    xt8_r = xt8_ap.rearrange("(ct p) t -> p ct t", p=128)

    with tile.TileContext(nc) as tc, ExitStack() as ctx:
        pers = ctx.enter_context(tc.tile_pool(name="pers", bufs=1))
        ident_sb = pers.tile([128, 128], BF16)
        # diagonal-block causal mask [s, par, tb]: 1.0 where s <= tb
        amask_sb = pers.tile([128, 2, 128], BF16)
        bias_sb = pers.tile([1, C], F32R)
        ones_row = pers.tile([1, 128], F32R)
        xT = pers.tile([128, 8, T], BF16)      # x^T [c-in-ct, ct, t]
        xT8 = pers.tile([128, 8, T], FP8)
        # v stationary [s-in-tile, tt, h, 128]: cols 0-63 = v_h, 64-127 = 1.0
        v_sb = pers.tile([128, 16, 8, 128], BF16)
        # kq8: qk-proj PSUM drain [par*64+dh*32+q, pair, kq, t]
        kq8 = pers.tile([128, NP, 2, T], FP8 if FP8_S else BF16)
        if FP8_S:
            # DoubleRow layout: [par*32+q, dh, pair, kq, t]
            kqT8 = pers.tile([64, 2, NP, 2, T], FP8)
        out_T = pers.tile([128, 4, T], BF16)

        make_identity(nc, ident_sb)
        nc.scalar.activation(ones_row, ident_sb[0:1, :], AF.Copy,
                             bias=1.0, scale=0.0)
        nc.gpsimd.memset(amask_sb, 1.0)
        # keep 1.0 where tb - s >= 0 (causal), else fill 0.0
        nc.gpsimd.affine_select(amask_sb, amask_sb,
                                pattern=[[0, 2], [1, 128]],
                                compare_op=ALU.is_ge, fill=0.0,
                                base=0, channel_multiplier=-1)
        # ones half of the PV stationary: only tts 0-3 are needed for round
        # 0; the rest is folded into the per-tt v fill items. On gpsimd so
        # the DVE queue head stays free for the latency-critical kq8 drains
        nc.gpsimd.memset(v_sb[:, 0:4, :, 64:128], 1.0)

        for _rep in range(repeat):
            with tc.tile_pool(name="wv", bufs=1) as wv_pool, \
                 tc.tile_pool(name="wqk", bufs=2) as wqk_pool, \
                 tc.tile_pool(name="pp", bufs=4) as p_pool, \
                 tc.tile_pool(name="nrm", bufs=2) as n_pool, \
                 tc.tile_pool(name="ostg", bufs=2) as o_pool, \
                 tc.tile_pool(name="psA", bufs=2, space="PSUM") as psA, \
                 tc.tile_pool(name="pss", bufs=2, space="PSUM") as ps_s, \
                 tc.tile_pool(name="pspv", bufs=1, space="PSUM") as ps_pv:

                # ---- Phase 1: only round-0-critical loads up front; the
                # rest of x and the weights stream in as round fill so the
                # serialized DMA device serves the first remaps fast ----
                nc.scalar.dma_start(bias_sb, bias_ap.bitcast(F32R))
                wqk_sb = wqk_pool.tile([128, NP, 2, 8, 128],
                                       FP8 if FP8_QK else BF16)
                wqk_r = wqk_ap.rearrange("(pr kq ct p) m -> p pr kq ct m",
                                         pr=NP, kq=2, p=128)
                nc.scalar.dma_start(wqk_sb[:, 0], wqk_r[:, 0])
                nc.sync.dma_start(xT8[:, :, 0:512], xt8_r[:, :, 0:512])
                nc.sync.dma_start(xT[:, :, 0:512], xt_r[:, :, 0:512])
                wv_sb = wv_pool.tile([128, 8, W], BF16)
                nc.sync.dma_start(
                    wv_sb, wv_ap.rearrange("(ct p) n -> p ct n", p=128))
                for pr in range(1, NP):
                    nc.sync.dma_start(wqk_sb[:, pr], wqk_r[:, pr])
                wp_sb = wv_pool.tile([128, 4, C], BF16)

                def load_x(g):
                    """stream in the g-th t-quarter of x^T (both dtypes)"""
                    def f():
                        cs = slice(g * 512, (g + 1) * 512)
                        nc.sync.dma_start(xT8[:, :, cs], xt8_r[:, :, cs])
                        nc.sync.dma_start(xT[:, :, cs], xt_r[:, :, cs])
                    return f
                bias_bc = wv_pool.tile([128, C], BF16)

                def emit_bias_bc():
                    # bias broadcast [128, C] via PE (2 matmuls) + DVE copy
                    for ch in range(2):
                        pbb = psA.tile([128, 512], F32, tag="scr",
                                       name="pbb")
                        nc.tensor.matmul(pbb, ones_row,
                                         bias_sb[:, ch * 512:(ch + 1) * 512],
                                         start=True, stop=True)
                        nc.vector.tensor_copy(
                            bias_bc[:, ch * 512:(ch + 1) * 512], pbb)

                def emit_qk_chunk(pair, kq, tc4):
                    """q/k projection for one (kq, 512-col t window):
                    4 DoubleRow matmuls -> PSUM, DVE drain (fp8 quantize),
                    SWDGE remap DMA into the DoubleRow k-tile layout."""
                    cs = slice(tc4 * 512, (tc4 + 1) * 512)
                    pqk = psA.tile([128, 512], F32, tag="scr")
                    if FP8_QK:
                        for g in range(4):
                            nc.tensor.matmul(
                                pqk,
                                wqk_sb[:, pair, kq, 2 * g:2 * g + 2, :],
                                xT8[:, 2 * g:2 * g + 2, cs],
                                start=(g == 0), stop=(g == 3),
                                perf_mode=DRMODE)
                    else:
                        for ct in range(8):
                            nc.tensor.matmul(
                                pqk, wqk_sb[:, pair, kq, ct, :],
                                xT[:, ct, cs],
                                start=(ct == 0), stop=(ct == 7))
                    nc.vector.tensor_copy(kq8[:, pair, kq, cs], pqk)
                    if FP8_S:
                        # plain partition-base-shifted copies (only dim0 of
                        # an AP may address partitions). par0 on SWDGE
                        # (Pool), par1 on scalar HWDGE: halves the serial
                        # per-queue remap latency; both queues are free of
                        # bulk loads
                        for par in range(2):
                            for dh in range(2):
                                p0 = par * 64 + dh * 32
                                src = kq8[p0:p0 + 32, pair, kq, cs]
                                dst = kqT8[par * 32:(par + 1) * 32, dh,
                                           pair, kq, cs]
                                eng = nc.gpsimd if par == 0 else nc.scalar
                                eng.dma_start(dst, src)

                def emit_v_tt(tt):
                    """v = x @ Wv for one 128-token tile."""
                    pv_ = psA.tile([128, W], F32, tag="scr")
                    for ct in range(8):
                        nc.tensor.matmul(
                            pv_,
                            xT[:, ct, tt * 128:(tt + 1) * 128],
                            wv_sb[:, ct, :],
                            start=(ct == 0), stop=(ct == 7))
                    nc.vector.tensor_copy(v_sb[:, tt, :, 0:64], pv_)

                def v_tt_items(tt):
                    """emit_v_tt split into two ~0.9us fill items sharing
                    one PSUM accumulation group."""
                    cell = {}

                    def first_half():
                        cell["t"] = psA.tile([128, W], F32, tag="scr",
                                             name="pv_")
                        for ct in range(4):
                            nc.tensor.matmul(
                                cell["t"],
                                xT[:, ct, tt * 128:(tt + 1) * 128],
                                wv_sb[:, ct, :],
                                start=(ct == 0), stop=False)

                    def second_half():
                        for ct in range(4, 8):
                            nc.tensor.matmul(
                                cell["t"],
                                xT[:, ct, tt * 128:(tt + 1) * 128],
                                wv_sb[:, ct, :],
                                start=False, stop=(ct == 7))
                        nc.vector.tensor_copy(v_sb[:, tt, :, 0:64],
                                              cell["t"])
                        nc.gpsimd.memset(v_sb[:, tt, :, 64:128], 1.0)

                    return [(860, first_half), (860, second_half)]

                # attention, software-pipelined one step ahead: for global
                # step order [... (pair,tc4,st) ...], S+exp(step i+1) is
                # emitted BEFORE PV(step i) so the Act engine always has
                # the next exp in flight while PE runs PV and fill items.
                p_t_q = []     # FIFO of p_t tiles: S/exp producer -> PV
                blk_pvl = {}   # (pair, tc4) -> pvl PSUM pair

                def emit_S_exp(pair, tc4, st):
                    ps = ps_s.tile([128, 2, 512], F32)
                    j = st - 4 * tc4
                    c0 = max(j, 0) * 128
                    for par in range(2):
                        if FP8_S and tc4 == 0:
                            # round 0: direct (non-DoubleRow) fp8 S from
                            # kq8 -- 2x matmul cost for these 4 sts, but
                            # keeps the remap DMA round-trip off the
                            # first-exp critical path
                            nc.tensor.matmul(
                                ps[:, par, c0:],
                                kq8[par * 64:par * 64 + 64, pair, 0,
                                    st * 128:(st + 1) * 128],
                                kq8[par * 64:par * 64 + 64, pair, 1,
                                    c0:512],
                                start=True, stop=True)
                        elif FP8_S:
                            nc.tensor.matmul(
                                ps[:, par, c0:],
                                kqT8[par * 32:(par + 1) * 32, :, pair,
                                     0, st * 128:(st + 1) * 128],
                                kqT8[par * 32:(par + 1) * 32, :, pair,
                                     1, tc4 * 512 + c0:(tc4 + 1) * 512],
                                start=True, stop=True,
                                perf_mode=DRMODE)
                        else:
                            nc.tensor.matmul(
                                ps[:, par, c0:],
                                kq8[par * 64:par * 64 + 64, pair, 0,
                                    st * 128:(st + 1) * 128],
                                kq8[par * 64:par * 64 + 64, pair, 1,
                                    tc4 * 512 + c0:(tc4 + 1) * 512],
                                start=True, stop=True)
                    p_t = p_pool.tile([128, 2, 512], BF16)
                    nc.scalar.activation(p_t[:, :, c0:],
                                         ps[:, :, c0:], AF.Exp,
                                         bias=0.0, scale=exp_scal)
                    if j >= 0:
                        # diagonal block: multiplicative tril mask
                        nc.vector.tensor_tensor(
                            p_t[:, :, c0:c0 + 128],
                            p_t[:, :, c0:c0 + 128],
                            amask_sb,
                            ALU.mult)
                    p_t_q.append(p_t)

                def emit_PV(pair, tc4, st):
                    n_s = 4 * (tc4 + 1)
                    c0 = max(st - 4 * tc4, 0) * 128
                    if st == 0:
                        blk_pvl[(pair, tc4)] = [
                            ps_pv.tile([128, 512], F32, name=f"pvl{par}")
                            for par in range(2)]
                    pvl = blk_pvl[(pair, tc4)]
                    p_t = p_t_q.pop(0)
                    st_f, st_l = (st == 0), (st == n_s - 1)
                    for par in range(2):
                        h = 2 * pair + par
                        # [v_h | ones] stationary: PSUM rows 0-63
                        # accumulate P@V, rows 64-127 the row-sum l
                        nc.tensor.matmul(
                            pvl[par][:, c0:],
                            v_sb[:, st, h, :],
                            p_t[:, par, c0:],
                            start=st_f, stop=st_l,
                            skip_group_check=True)
                    if st_l:
                        pvl = blk_pvl.pop((pair, tc4))
                        rcl = n_pool.tile([128, 512], F32)
                        for par in range(2):
                            nc.vector.reciprocal(
                                rcl[par * 64:par * 64 + 64],
                                pvl[par][64:128])
                        for par in range(2):
                            nc.vector.scalar_tensor_tensor(
                                out_T[par * 64:par * 64 + 64, pair,
                                      tc4 * 512:(tc4 + 1) * 512],
                                pvl[par][0:64], 1.0,
                                rcl[par * 64:par * 64 + 64],
                                ALU.mult, ALU.mult)

                out_r = out_ap.rearrange("(tt p) n -> p tt n", p=128)

                def proj_tt_items(tt):
                    """output projection for one 128-token tile as two
                    ~1.5us fill items (one per 512-chan half). GPSIMD has
                    no PSUM port, so the bias add stays on DVE."""
                    cell = {}

                    def mk(ch):
                        def f():
                            if ch == 0:
                                cell["o"] = o_pool.tile([128, C], F32,
                                                        name="ostg")
                            po = psA.tile([128, 512], F32, tag="scr")
                            for ct in range(4):
                                nc.tensor.matmul(
                                    po,
                                    out_T[:, ct, tt * 128:(tt + 1) * 128],
                                    wp_sb[:, ct, ch * 512:(ch + 1) * 512],
                                    start=(ct == 0), stop=(ct == 3))
                            nc.vector.tensor_tensor(
                                cell["o"][:, ch * 512:(ch + 1) * 512], po,
                                bias_bc[:, ch * 512:(ch + 1) * 512],
                                ALU.add)
                            if ch == 1:
                                nc.sync.dma_start(out_r[:, tt, :],
                                                  cell["o"])
                        return f

                    return [(860, mk(0)), (860, mk(1))]

                def qk_items(pair, r):
                    return [(430, lambda p=pair, kq=kq, r=r:
                             emit_qk_chunk(p, kq, r)) for kq in range(2)]

                def proj_round_items(r):
                    out = []
                    for tt in range(4 * r, 4 * r + 4):
                        out += proj_tt_items(tt)
                    return out

                # ---- Phase 2+3+4: tc4-round-major, software-pipelined ----
                def round_fill(r):
                    fill = []
                    if r == 0:
                        # round 0 still owes pairs 1-3 their r=0 chunks
                        # (front of the queue: block p is 4 sts away)
                        for p in range(1, NP):
                            fill += qk_items(p, 0)
                    if r == 1:
                        # wp load: needed from round 2's proj fill on
                        fill += [(0, lambda: nc.scalar.dma_start(
                            wp_sb,
                            wp_ap.rearrange("(ct p) n -> p ct n", p=128)))]
                    if r < 3:
                        # next round's inputs: the x quarter load (free on
                        # PE), qk chunks for all pairs, then v tiles
                        fill += [(0, load_x(r + 1))]
                        for p in range(NP):
                            fill += qk_items(p, r + 1)
                        for tt in range(4 * (r + 1), 4 * (r + 1) + 4):
                            fill += v_tt_items(tt)
                    if r == 2:
                        fill += proj_round_items(0)
                    elif r == 3:
                        fill += proj_round_items(1) + proj_round_items(2)
                    return fill

                # global step list (pair-blocks within tc4-rounds) and the
                # per-round fill queues, drawn after each PV by progress
                steps = [(p, r, st)
                         for r in range(4) for p in range(NP)
                         for st in range(4 * (r + 1))]
                fills = {r: round_fill(r) for r in range(4)}
                totals = {r: sum(c for c, _ in fills[r]) or 1
                          for r in range(4)}
                n_sts = {r: NP * 4 * (r + 1) for r in range(4)}
                fstate = {r: {"sts": 0, "done": 0.0, "idx": 0}
                          for r in range(4)}

                def draw(r):
                    st8 = fstate[r]
                    fill = fills[r]
                    st8["sts"] += 1
                    while (st8["idx"] < len(fill)
                           and st8["done"] / totals[r]
                           <= st8["sts"] / n_sts[r]):
                        c, fn = fill[st8["idx"]]
                        fn()
                        st8["done"] += max(c, 1)
                        st8["idx"] += 1

                def flush(r):
                    st8 = fstate[r]
                    while st8["idx"] < len(fills[r]):
                        fills[r][st8["idx"]][1]()
                        st8["idx"] += 1

                emit_bias_bc()  # also warms the PE pstate early
                for it in qk_items(0, 0):
                    it[1]()
                emit_S_exp(*steps[0])
                for tt in range(4):
                    emit_v_tt(tt)

                for i, (p, r, st) in enumerate(steps):
                    if i + 1 < len(steps):
                        emit_S_exp(*steps[i + 1])
                    emit_PV(p, r, st)
                    draw(r)
                    if (p, st) == (NP - 1, 4 * (r + 1) - 1):
                        flush(r)
                for it in proj_round_items(3):
                    it[1]()

    nc.finalize()
    return nc


def _in_maps(inputs):
    import ml_dtypes
    bf16 = ml_dtypes.bfloat16
    fp8 = ml_dtypes.float8_e4m3
    x = np.asarray(inputs["x"], dtype=np.float32)
    Wq = np.asarray(inputs["Wq"], dtype=np.float32)
    Wk = np.asarray(inputs["Wk"], dtype=np.float32)
    Wv = np.asarray(inputs["Wv"], dtype=np.float32)
    Wp = np.asarray(inputs["Wproj"], dtype=np.float32)
    bp = np.asarray(inputs["bproj"], dtype=np.float32)

    maps = []
    for core in range(8):
        b, hh = core // 2, core % 2
        hs0 = hh * HL
        # WQK [pair, kq, C, 128]: kq=0 Wk pair, kq=1 Wq pair
        wqk = np.empty((NP, 2, C, 128), dtype=np.float32)
        for pr in range(NP):
            he, ho = hs0 + 2 * pr, hs0 + 2 * pr + 1
            wqk[pr, 0, :, 0:64] = Wk[he]
            wqk[pr, 0, :, 64:128] = Wk[ho]
            wqk[pr, 1, :, 0:64] = Wq[he]
            wqk[pr, 1, :, 64:128] = Wq[ho]
        wqk = wqk.reshape(NP * 2 * C, 128)
        wqk = (wqk * 16.0).astype(fp8) if FP8_QK else wqk.astype(bf16)
        wv = Wv[hs0:hs0 + HL].transpose(1, 0, 2).reshape(C, W)
        wv = wv.astype(bf16)
        wp = Wp[hh * W:(hh + 1) * W].astype(bf16)
        bias = (bp if hh == 0 else np.zeros_like(bp)).reshape(1, C)
        xt = np.ascontiguousarray(x[b].T)
        maps.append({
            "XT": np.ascontiguousarray(xt.astype(bf16)),
            "XT8": np.ascontiguousarray(xt.astype(fp8)),
            "WQK": np.ascontiguousarray(wqk),
            "WV": np.ascontiguousarray(wv),
            "WP": np.ascontiguousarray(wp),
            "BIAS": np.ascontiguousarray(bias),
        })
    return maps


def get_nc(repeat=1):
    key = repeat
    if key not in _NC:
        _NC[key] = _build(repeat)
    return _NC[key]


def run(inputs, trace=False):
    res = bass_utils.run_bass_kernel_spmd(
        get_nc(), _in_maps(inputs), core_ids=list(range(8)), trace=trace)
    outs = [res.results[c]["OUT"] for c in range(8)]
    out = np.stack([outs[2 * b] + outs[2 * b + 1] for b in range(B)])
    return out.astype(np.float32), res.exec_time_ns


def kernel(**inputs):
    return run(inputs, trace=False)[0]


# revision 49
# speedup vs baseline: 1.0888x; 1.0888x over previous
import sys
sys.path.insert(0, "/opt/trn_rl_repo")
import numpy as np
from contextlib import ExitStack

from concourse import bacc, mybir, tile
from concourse import bass_utils
from concourse.masks import make_identity

# nn_MultiHeadAttention: B=4, T=2048, C=1024, H=16, HS=64
# Sharding: core = 2*b + hh; each core handles batch b, heads hh*8..hh*8+7.
# Per-core output is a partial [T, C] (its 8 heads through Wproj rows);
# host sums the pair (hh=0,1) per batch. Bias folded into hh==0 cores.
#
# v7 (from v2 baseline):
#  - l (softmax denominator) fused into the PV matmul: stationary widened
#    to [v_h | ones] so PSUM rows 64-127 accumulate the row-sum while rows
#    0-63 accumulate P@V. Matmul cost is moving-column count, so l is free;
#    v2's separate ones-stationary l matmuls are gone.
#  - S = K^T Q in fp8 DoubleRow: q/k tiles quantized to fp8 on the PSUM
#    drain, then SWDGE SB->SB DMAs remap [par*64+dh*32+q] partitions into
#    [par*32+q, dh] (k-tile pairs in the free dim) as DoubleRow requires.
#  - x arrives pre-transposed (and pre-cast to fp8) from the host: no XBAR
#    transposes, no on-device fp8 casts of x.
#  - causal mask built on-device (one affine_select tril; the mask is only
#    ever applied to the 128-wide diagonal block, where it is j-independent).
#  - emission is tc4-round-major and software-pipelined: engines run their
#    streams in order, so independent PE work (next round's qk/v prep,
#    finished rounds' output proj) is spread as small fill items drawn
#    after each attention key-tile to fill the exp-wait stalls.

B, T, C = 4, 2048, 1024
H, HS = 16, 64
HL = 8            # local heads per core
NP = HL // 2      # head pairs per core
W = HL * HS       # 512
SCALE = 1.0 / 32.0  # C ** -0.5

FP8_QK = True     # x8/wqk8 -> q,k projection path
FP8_S = False     # fp8 DoubleRow for S = K^T Q (kq stored fp8)

F32 = mybir.dt.float32
F32R = mybir.dt.float32r
BF16 = mybir.dt.bfloat16
FP8 = mybir.dt.float8e4
DRMODE = mybir.MatmulPerfMode.DoubleRow
AF = mybir.ActivationFunctionType
ALU = mybir.AluOpType

_NC = {}


def _build(repeat=1):
    exp_scal = SCALE / (256.0 if FP8_QK else 1.0)

    nc = bacc.Bacc("TRN2", target_bir_lowering=False, debug=False, num_devices=1)
    # x^T, host-pretransposed: [C, T] in bf16 (v path) and fp8 (qk path)
    xt_ap = nc.dram_tensor("XT", (C, T), BF16, kind="ExternalInput").ap()
    xt8_ap = nc.dram_tensor("XT8", (C, T), FP8, kind="ExternalInput").ap()
    # WQK [pair, kq, C, 128]; kq=0 -> [Wk_even|Wk_odd], kq=1 -> [Wq_even|Wq_odd]
    wqk_ap = nc.dram_tensor("WQK", (NP * 2 * C, 128),
                            FP8 if FP8_QK else BF16,
                            kind="ExternalInput").ap()
    wv_ap = nc.dram_tensor("WV", (C, W), BF16, kind="ExternalInput").ap()
    wp_ap = nc.dram_tensor("WP", (W, C), BF16, kind="ExternalInput").ap()
    bias_ap = nc.dram_tensor("BIAS", (1, C), F32, kind="ExternalInput").ap()
    out_ap = nc.dram_tensor("OUT", (T, C), F32, kind="ExternalOutput").ap()

    xt_r = xt_ap.rearrange("(ct p) t -> p ct t", p=128)
    xt8_r = xt8_ap.rearrange("(ct p) t -> p ct t", p=128)

    with tile.TileContext(nc) as tc, ExitStack() as ctx:
        pers = ctx.enter_context(tc.tile_pool(name="pers", bufs=1))
        ident_sb = pers.tile([128, 128], BF16)
        # diagonal-block causal mask [s, par, tb]: 1.0 where s <= tb
        amask_sb = pers.tile([128, 2, 128], BF16)
        bias_sb = pers.tile([1, C], F32R)
        ones_row = pers.tile([1, 128], F32R)
        xT = pers.tile([128, 8, T], BF16)      # x^T [c-in-ct, ct, t]
        xT8 = pers.tile([128, 8, T], FP8)
        # v stationary [s-in-tile, tt, h, 128]: cols 0-63 = v_h, 64-127 = 1.0
        v_sb = pers.tile([128, 16, 8, 128], BF16)
        # kq8: qk-proj PSUM drain [par*64+dh*32+q, pair, kq, t]
        kq8 = pers.tile([128, NP, 2, T], FP8 if FP8_S else BF16)
        if FP8_S:
            # DoubleRow layout: [par*32+q, dh, pair, kq, t]
            kqT8 = pers.tile([64, 2, NP, 2, T], FP8)
        out_T = pers.tile([128, 4, T], BF16)

        make_identity(nc, ident_sb)
        nc.scalar.activation(ones_row, ident_sb[0:1, :], AF.Copy,
                             bias=1.0, scale=0.0)
        nc.gpsimd.memset(amask_sb, 1.0)
        # keep 1.0 where tb - s >= 0 (causal), else fill 0.0
        nc.gpsimd.affine_select(amask_sb, amask_sb,
                                pattern=[[0, 2], [1, 128]],
                                compare_op=ALU.is_ge, fill=0.0,
                                base=0, channel_multiplier=-1)
        # ones half of the PV stationary: only tts 0-3 are needed for round
        # 0; the rest is folded into the per-tt v fill items. On gpsimd so
        # the DVE queue head stays free for the latency-critical kq8 drains
        nc.gpsimd.memset(v_sb[:, 0:4, :, 64:128], 1.0)

        for _rep in range(repeat):
            with tc.tile_pool(name="wv", bufs=1) as wv_pool, \
                 tc.tile_pool(name="wqk", bufs=2) as wqk_pool, \
                 tc.tile_pool(name="pp", bufs=4) as p_pool, \
                 tc.tile_pool(name="nrm", bufs=2) as n_pool, \
                 tc.tile_pool(name="ostg", bufs=2) as o_pool, \
                 tc.tile_pool(name="psA", bufs=2, space="PSUM") as psA, \
                 tc.tile_pool(name="pss", bufs=2, space="PSUM") as ps_s, \
                 tc.tile_pool(name="pspv", bufs=1, space="PSUM") as ps_pv:

                # ---- Phase 1: only round-0-critical loads up front; the
                # rest of x and the weights stream in as round fill so the
                # serialized DMA device serves the first remaps fast ----
                nc.scalar.dma_start(bias_sb, bias_ap.bitcast(F32R))
                wqk_sb = wqk_pool.tile([128, NP, 2, 8, 128],
                                       FP8 if FP8_QK else BF16)
                wqk_r = wqk_ap.rearrange("(pr kq ct p) m -> p pr kq ct m",
                                         pr=NP, kq=2, p=128)
                nc.scalar.dma_start(wqk_sb[:, 0], wqk_r[:, 0])
                nc.sync.dma_start(xT8[:, :, 0:512], xt8_r[:, :, 0:512])
                nc.sync.dma_start(xT[:, :, 0:512], xt_r[:, :, 0:512])
                wv_sb = wv_pool.tile([128, 8, W], BF16)
                nc.sync.dma_start(
                    wv_sb, wv_ap.rearrange("(ct p) n -> p ct n", p=128))
                for pr in range(1, NP):
                    nc.sync.dma_start(wqk_sb[:, pr], wqk_r[:, pr])
                wp_sb = wv_pool.tile([128, 4, C], BF16)

                def load_x(g):
                    """stream in the g-th t-quarter of x^T (both dtypes)"""
                    def f():
                        cs = slice(g * 512, (g + 1) * 512)
                        nc.sync.dma_start(xT8[:, :, cs], xt8_r[:, :, cs])
                        nc.sync.dma_start(xT[:, :, cs], xt_r[:, :, cs])
                    return f
                bias_bc = wv_pool.tile([128, C], BF16)

                def emit_bias_bc():
                    # bias broadcast [128, C] via PE (2 matmuls) + DVE copy
                    for ch in range(2):
                        pbb = psA.tile([128, 512], F32, tag="scr",
                                       name="pbb")
                        nc.tensor.matmul(pbb, ones_row,
                                         bias_sb[:, ch * 512:(ch + 1) * 512],
                                         start=True, stop=True)
                        nc.vector.tensor_copy(
                            bias_bc[:, ch * 512:(ch + 1) * 512], pbb)

                def emit_qk_chunk(pair, kq, tc4):
                    """q/k projection for one (kq, 512-col t window):
                    4 DoubleRow matmuls -> PSUM, DVE drain (fp8 quantize),
                    SWDGE remap DMA into the DoubleRow k-tile layout."""
                    cs = slice(tc4 * 512, (tc4 + 1) * 512)
                    pqk = psA.tile([128, 512], F32, tag="scr")
                    if FP8_QK:
                        for g in range(4):
                            nc.tensor.matmul(
                                pqk,
                                wqk_sb[:, pair, kq, 2 * g:2 * g + 2, :],
                                xT8[:, 2 * g:2 * g + 2, cs],
                                start=(g == 0), stop=(g == 3),
                                perf_mode=DRMODE)
                    else:
                        for ct in range(8):
                            nc.tensor.matmul(
                                pqk, wqk_sb[:, pair, kq, ct, :],
                                xT[:, ct, cs],
                                start=(ct == 0), stop=(ct == 7))
                    nc.vector.tensor_copy(kq8[:, pair, kq, cs], pqk)
                    if FP8_S:
                        # plain partition-base-shifted copies (only dim0 of
                        # an AP may address partitions). par0 on SWDGE
                        # (Pool), par1 on scalar HWDGE: halves the serial
                        # per-queue remap latency; both queues are free of
                        # bulk loads
                        for par in range(2):
                            for dh in range(2):
                                p0 = par * 64 + dh * 32
                                src = kq8[p0:p0 + 32, pair, kq, cs]
                                dst = kqT8[par * 32:(par + 1) * 32, dh,
                                           pair, kq, cs]
                                eng = nc.gpsimd if par == 0 else nc.scalar
                                eng.dma_start(dst, src)

                def emit_v_tt(tt):
                    """v = x @ Wv for one 128-token tile."""
                    pv_ = psA.tile([128, W], F32, tag="scr")
                    for ct in range(8):
                        nc.tensor.matmul(
                            pv_,
                            xT[:, ct, tt * 128:(tt + 1) * 128],
                            wv_sb[:, ct, :],
                            start=(ct == 0), stop=(ct == 7))
                    nc.vector.tensor_copy(v_sb[:, tt, :, 0:64], pv_)

                def v_tt_items(tt):
                    """emit_v_tt split into two ~0.9us fill items sharing
                    one PSUM accumulation group."""
                    cell = {}

                    def first_half():
                        cell["t"] = psA.tile([128, W], F32, tag="scr",
                                             name="pv_")
                        for ct in range(4):
                            nc.tensor.matmul(
                                cell["t"],
                                xT[:, ct, tt * 128:(tt + 1) * 128],
                                wv_sb[:, ct, :],
                                start=(ct == 0), stop=False)

                    def second_half():
                        for ct in range(4, 8):
                            nc.tensor.matmul(
                                cell["t"],
                                xT[:, ct, tt * 128:(tt + 1) * 128],
                                wv_sb[:, ct, :],
                                start=False, stop=(ct == 7))
                        nc.vector.tensor_copy(v_sb[:, tt, :, 0:64],
                                              cell["t"])
                        nc.gpsimd.memset(v_sb[:, tt, :, 64:128], 1.0)

                    return [(860, first_half), (860, second_half)]

                # attention, software-pipelined one step ahead: for global
                # step order [... (pair,tc4,st) ...], S+exp(step i+1) is
                # emitted BEFORE PV(step i) so the Act engine always has
                # the next exp in flight while PE runs PV and fill items.
                p_t_q = []     # FIFO of p_t tiles: S/exp producer -> PV
                blk_pvl = {}   # (pair, tc4) -> pvl PSUM pair

                def emit_S_exp(pair, tc4, st):
                    ps = ps_s.tile([128, 2, 512], F32)
                    j = st - 4 * tc4
                    c0 = max(j, 0) * 128
                    for par in range(2):
                        if FP8_S and tc4 == 0:
                            # round 0: direct (non-DoubleRow) fp8 S from
                            # kq8 -- 2x matmul cost for these 4 sts, but
                            # keeps the remap DMA round-trip off the
                            # first-exp critical path
                            nc.tensor.matmul(
                                ps[:, par, c0:],
                                kq8[par * 64:par * 64 + 64, pair, 0,
                                    st * 128:(st + 1) * 128],
                                kq8[par * 64:par * 64 + 64, pair, 1,
                                    c0:512],
                                start=True, stop=True)
                        elif FP8_S:
                            nc.tensor.matmul(
                                ps[:, par, c0:],
                                kqT8[par * 32:(par + 1) * 32, :, pair,
                                     0, st * 128:(st + 1) * 128],
                                kqT8[par * 32:(par + 1) * 32, :, pair,
                                     1, tc4 * 512 + c0:(tc4 + 1) * 512],
                                start=True, stop=True,
                                perf_mode=DRMODE)
                        else:
                            nc.tensor.matmul(
                                ps[:, par, c0:],
                                kq8[par * 64:par * 64 + 64, pair, 0,
                                    st * 128:(st + 1) * 128],
                                kq8[par * 64:par * 64 + 64, pair, 1,
                                    tc4 * 512 + c0:(tc4 + 1) * 512],
                                start=True, stop=True)
                    p_t = p_pool.tile([128, 2, 512], BF16)
                    nc.scalar.activation(p_t[:, :, c0:],
                                         ps[:, :, c0:], AF.Exp,
                                         bias=0.0, scale=exp_scal)
                    if j >= 0:
                        # diagonal block: multiplicative tril mask
                        nc.vector.tensor_tensor(
                            p_t[:, :, c0:c0 + 128],
                            p_t[:, :, c0:c0 + 128],
                            amask_sb,
                            ALU.mult)
                    p_t_q.append(p_t)

                def emit_PV(pair, tc4, st):
                    n_s = 4 * (tc4 + 1)
                    c0 = max(st - 4 * tc4, 0) * 128
                    if st == 0:
                        blk_pvl[(pair, tc4)] = [
                            ps_pv.tile([128, 512], F32, name=f"pvl{par}")
                            for par in range(2)]
                    pvl = blk_pvl[(pair, tc4)]
                    p_t = p_t_q.pop(0)
                    st_f, st_l = (st == 0), (st == n_s - 1)
                    for par in range(2):
                        h = 2 * pair + par
                        # [v_h | ones] stationary: PSUM rows 0-63
                        # accumulate P@V, rows 64-127 the row-sum l
                        nc.tensor.matmul(
                            pvl[par][:, c0:],
                            v_sb[:, st, h, :],
                            p_t[:, par, c0:],
                            start=st_f, stop=st_l,
                            skip_group_check=True)
                    if st_l:
                        pvl = blk_pvl.pop((pair, tc4))
                        rcl = n_pool.tile([128, 512], F32)
                        for par in range(2):
                            nc.vector.reciprocal(
                                rcl[par * 64:par * 64 + 64],
                                pvl[par][64:128])
                        for par in range(2):
                            nc.vector.scalar_tensor_tensor(
                                out_T[par * 64:par * 64 + 64, pair,
                                      tc4 * 512:(tc4 + 1) * 512],
                                pvl[par][0:64], 1.0,
                                rcl[par * 64:par * 64 + 64],
                                ALU.mult, ALU.mult)

                out_r = out_ap.rearrange("(tt p) n -> p tt n", p=128)

                def proj_tt_items(tt):
                    """output projection for one 128-token tile as two
                    ~1.5us fill items (one per 512-chan half). GPSIMD has
                    no PSUM port, so the bias add stays on DVE."""
                    cell = {}

                    def mk(ch):
                        def f():
                            if ch == 0:
                                cell["o"] = o_pool.tile([128, C], F32,
                                                        name="ostg")
                            po = psA.tile([128, 512], F32, tag="scr")
                            for ct in range(4):
                                nc.tensor.matmul(
                                    po,
                                    out_T[:, ct, tt * 128:(tt + 1) * 128],
                                    wp_sb[:, ct, ch * 512:(ch + 1) * 512],
                                    start=(ct == 0), stop=(ct == 3))
                            nc.vector.tensor_tensor(
                                cell["o"][:, ch * 512:(ch + 1) * 512], po,
                                bias_bc[:, ch * 512:(ch + 1) * 512],
                                ALU.add)
                            if ch == 1:
                                nc.sync.dma_start(out_r[:, tt, :],
                                                  cell["o"])
                        return f

                    return [(860, mk(0)), (860, mk(1))]

                def qk_items(pair, r):
                    return [(430, lambda p=pair, kq=kq, r=r:
                             emit_qk_chunk(p, kq, r)) for kq in range(2)]

                def proj_round_items(r):
                    out = []
                    for tt in range(4 * r, 4 * r + 4):
                        out += proj_tt_items(tt)
                    return out

                # ---- Phase 2+3+4: tc4-round-major, software-pipelined ----
                def round_fill(r):
                    fill = []
                    if r == 0:
                        # round 0 still owes pairs 1-3 their r=0 chunks
                        # (front of the queue: block p is 4 sts away)
                        for p in range(1, NP):
                            fill += qk_items(p, 0)
                    if r == 1:
                        # wp load: needed from round 2's proj fill on
                        fill += [(0, lambda: nc.scalar.dma_start(
                            wp_sb,
                            wp_ap.rearrange("(ct p) n -> p ct n", p=128)))]
                    if r < 3:
                        # next round's inputs: the x quarter load (free on
                        # PE), qk chunks for all pairs, then v tiles
                        fill += [(0, load_x(r + 1))]
                        for p in range(NP):
                            fill += qk_items(p, r + 1)
                        for tt in range(4 * (r + 1), 4 * (r + 1) + 4):
                            fill += v_tt_items(tt)
                    if r == 2:
                        fill += proj_round_items(0)
                    elif r == 3:
                        fill += proj_round_items(1) + proj_round_items(2)
                    return fill

                # global step list (pair-blocks within tc4-rounds) and the
                # per-round fill queues, drawn after each PV by progress
                steps = [(p, r, st)
                         for r in range(4) for p in range(NP)
                         for st in range(4 * (r + 1))]
                fills = {r: round_fill(r) for r in range(4)}
                totals = {r: sum(c for c, _ in fills[r]) or 1
                          for r in range(4)}
                n_sts = {r: NP * 4 * (r + 1) for r in range(4)}
                fstate = {r: {"sts": 0, "done": 0.0, "idx": 0}
                          for r in range(4)}

                def draw(r):
                    st8 = fstate[r]
                    fill = fills[r]
                    st8["sts"] += 1
                    while (st8["idx"] < len(fill)
                           and st8["done"] / totals[r]
                           <= st8["sts"] / n_sts[r]):
                        c, fn = fill[st8["idx"]]
                        fn()
                        st8["done"] += max(c, 1)
                        st8["idx"] += 1

                def flush(r):
                    st8 = fstate[r]
                    while st8["idx"] < len(fills[r]):
                        fills[r][st8["idx"]][1]()
                        st8["idx"] += 1

                emit_bias_bc()  # also warms the PE pstate early
                for it in qk_items(0, 0):
                    it[1]()
                emit_S_exp(*steps[0])
                for tt in range(4):
                    emit_v_tt(tt)

                for i, (p, r, st) in enumerate(steps):
                    if i + 1 < len(steps):
                        emit_S_exp(*steps[i + 1])
                    emit_PV(p, r, st)
                    draw(r)
                    if (p, st) == (NP - 1, 4 * (r + 1) - 1):
                        flush(r)
                for it in proj_round_items(3):
                    it[1]()

    nc.finalize()
    return nc


def _in_maps(inputs):
    import ml_dtypes
    bf16 = ml_dtypes.bfloat16
    fp8 = ml_dtypes.float8_e4m3
    x = np.asarray(inputs["x"], dtype=np.float32)
    Wq = np.asarray(inputs["Wq"], dtype=np.float32)
    Wk = np.asarray(inputs["Wk"], dtype=np.float32)
    Wv = np.asarray(inputs["Wv"], dtype=np.float32)
    Wp = np.asarray(inputs["Wproj"], dtype=np.float32)
    bp = np.asarray(inputs["bproj"], dtype=np.float32)

    maps = []
    for core in range(8):
        b, hh = core // 2, core % 2
        hs0 = hh * HL
        # WQK [pair, kq, C, 128]: kq=0 Wk pair, kq=1 Wq pair
        wqk = np.empty((NP, 2, C, 128), dtype=np.float32)
        for pr in range(NP):
            he, ho = hs0 + 2 * pr, hs0 + 2 * pr + 1
            wqk[pr, 0, :, 0:64] = Wk[he]
            wqk[pr, 0, :, 64:128] = Wk[ho]
            wqk[pr, 1, :, 0:64] = Wq[he]
            wqk[pr, 1, :, 64:128] = Wq[ho]
        wqk = wqk.reshape(NP * 2 * C, 128)
        wqk = (wqk * 16.0).astype(fp8) if FP8_QK else wqk.astype(bf16)
        wv = Wv[hs0:hs0 + HL].transpose(1, 0, 2).reshape(C, W)
        wv = wv.astype(bf16)
        wp = Wp[hh * W:(hh + 1) * W].astype(bf16)
        bias = (bp if hh == 0 else np.zeros_like(bp)).reshape(1, C)
        xt = np.ascontiguousarray(x[b].T)
        maps.append({
            "XT": np.ascontiguousarray(xt.astype(bf16)),
            "XT8": np.ascontiguousarray(xt.astype(fp8)),
            "WQK": np.ascontiguousarray(wqk),
            "WV": np.ascontiguousarray(wv),
            "WP": np.ascontiguousarray(wp),
            "BIAS": np.ascontiguousarray(bias),
        })
    return maps


def get_nc(repeat=1):
    key = repeat
    if key not in _NC:
        _NC[key] = _build(repeat)
    return _NC[key]


def run(inputs, trace=False):
    res = bass_utils.run_bass_kernel_spmd(
        get_nc(), _in_maps(inputs), core_ids=list(range(8)), trace=trace)
    outs = [res.results[c]["OUT"] for c in range(8)]
    out = np.stack([outs[2 * b] + outs[2 * b + 1] for b in range(B)])
    return out.astype(np.float32), res.exec_time_ns


def kernel(**inputs):
    return run(inputs, trace=False)[0]
